# Initial kernel scaffold
#
"""Trainium2 Bass kernel for causal attention with bias.

Reference computation (per (b, h)):
    S = (Q @ K^T) * D**-0.5 + bias          # [N, N]
    S[j > i] = -FLT_MAX                     # causal mask
    P = softmax(S, axis=-1)
    out = P @ V                             # [N, D]

Shapes: B=4, H=16, N=1024, D=64, fp32. The 64 (b, h) pairs are sharded
8-per-core across 8 NeuronCores (head-parallel, no collectives).

Per-core kernel strategy (per head):
  - PE-transpose q, k into [D, N] layout (matmul against identity).
  - Loop over key blocks j (128 keys each). Exploit causality: only
    query blocks i >= j are computed.
  - The bias strip for (j, i>=j) is DMA'd in natural [qi, kj] layout and
    transposed by the PE into PSUM (is_transpose matmul, start=True).
    The QK^T matmul then ACCUMULATES on top (start=False), giving
    S^T = bias^T + K_j Q^T directly in PSUM, in transposed [kj, qi]
    layout.
  - Softmax skips the row-max subtraction (scores are O(10), fp32 exp
    has massive headroom), so exp(S^T) on the scalar engine is a single
    pass that also serves as the PSUM->SBUF move, yielding P^T in
    exactly the layout the AV matmul needs as its stationary/moving
    operand (contraction over keys).
  - The causal diagonal block is masked AFTER the exp by zeroing the
    strictly-lower triangle (kj > qi) with gpsimd affine_select.
  - V is augmented with a ones column; AV accumulates out^T[d, qi] and
    row 64 accumulates the softmax denominator for free.
  - PE transposes out^T back, DVE computes reciprocal of the
    denominators and scales, result DMA'd out.
"""

import numpy as np

import concourse.bass as bass
import concourse.mybir as mybir
import concourse.tile as tile
from concourse.masks import make_identity

N_CORES = 8
B, H, N, D = 4, 16, 1024, 64
HEADS_PER_CORE = (B * H) // N_CORES  # 8
SCALE = float(D) ** -0.5

F32 = mybir.dt.float32
F32R = mybir.dt.float32r


def _mm_dtype(ap, use_f32r):
    return ap.bitcast(F32R) if use_f32r else ap


def build_attention_bass(heads=HEADS_PER_CORE, n=N, d=D, use_f32r=True):
    """Build the per-core Bass module. All cores run the same program
    (SPMD) on their own head slice."""
    nt = n // 128  # number of 128-row/col blocks
    assert n % 128 == 0 and d == 64

    nc = bass.Bass("TRN2")

    q_dram = nc.dram_tensor("q", [heads, n, d], F32, kind="ExternalInput")
    k_dram = nc.dram_tensor("k", [heads, n, d], F32, kind="ExternalInput")
    v_dram = nc.dram_tensor("v", [heads, n, d], F32, kind="ExternalInput")
    bias_dram = nc.dram_tensor("bias", [heads, n, n], F32, kind="ExternalInput")
    out_dram = nc.dram_tensor("out", [heads, n, d], F32, kind="ExternalOutput")

    with tile.TileContext(nc) as tc:
        with (
            tc.tile_pool(name="const", bufs=1) as const_pool,
            tc.tile_pool(name="qk", bufs=2) as qk_pool,
            tc.tile_pool(name="biasp", bufs=4) as bias_pool,
            tc.tile_pool(name="pt", bufs=2) as p_pool,
            tc.tile_pool(name="outp", bufs=2) as out_pool,
            tc.tile_pool(name="st", bufs=2, space="PSUM") as st_pool,
            tc.tile_pool(name="acc", bufs=1, space="PSUM") as acc_pool,
            tc.tile_pool(name="scratch", bufs=1, space="PSUM") as scratch_pool,
        ):
            ident = const_pool.tile([128, 128], F32)
            make_identity(nc, ident)

            for h in range(heads):
                # ---- load q, k, v (natural layout) --------------------
                q_nat = qk_pool.tile([128, nt, d], F32, tag="nat")
                nc.sync.dma_start(
                    out=q_nat, in_=q_dram[h].rearrange("(t p) d -> p t d", p=128)
                )
                k_nat = qk_pool.tile([128, nt, d], F32, tag="nat2")
                nc.sync.dma_start(
                    out=k_nat, in_=k_dram[h].rearrange("(t p) d -> p t d", p=128)
                )
                # v with an extra ones column (denominator trick)
                vhat = qk_pool.tile([128, nt, d + 1], F32)
                nc.sync.dma_start(
                    out=vhat[:, :, 0:d],
                    in_=v_dram[h].rearrange("(t p) d -> p t d", p=128),
                )
                nc.gpsimd.memset(vhat[:, :, d : d + 1], 1.0)

                # ---- transpose q, k into [d, n] -----------------------
                qT_ps = scratch_pool.tile([d, n], F32, tag="tscratch")
                for t in range(nt):
                    nc.tensor.transpose(
                        qT_ps[:, t * 128 : (t + 1) * 128], q_nat[:, t, :], ident
                    )
                qT = qk_pool.tile([d, n], F32)
                # fold the attention scale into q while evacuating PSUM
                nc.vector.tensor_scalar_mul(qT, qT_ps, SCALE)

                kT_ps = scratch_pool.tile([d, n], F32, tag="tscratch")
                for t in range(nt):
                    nc.tensor.transpose(
                        kT_ps[:, t * 128 : (t + 1) * 128], k_nat[:, t, :], ident
                    )
                kT = qk_pool.tile([d, n], F32)
                nc.vector.tensor_copy(kT, kT_ps)

                # out^T accumulator [d+1, n]; row d is the denominator
                outT_ps = acc_pool.tile([d + 1, n], F32)

                # ---- main loop over key blocks j ----------------------
                for j in range(nt):
                    span = (nt - j) * 128  # query columns j*128 .. n
                    c0 = j * 128  # absolute first query column

                    # bias strip: natural [qi, i-block, kj] layout
                    bias_sb = bias_pool.tile([128, nt - j, 128], F32, tag="bias_sb")
                    nc.sync.dma_start(
                        out=bias_sb,
                        in_=bias_dram[h][c0:n, c0 : c0 + 128].rearrange(
                            "(i p) c -> p i c", p=128
                        ),
                    )

                    # S^T strip in PSUM: [kj, qi] for qi in [c0, n)
                    st_ps = st_pool.tile([128, span], F32, tag="st")

                    # bias^T via PE transpose (start accumulation group)
                    for ii in range(nt - j):
                        nc.tensor.matmul(
                            st_ps[:, ii * 128 : (ii + 1) * 128],
                            bias_sb[:, ii, :],
                            ident,
                            is_transpose=True,
                            start=True,
                            stop=False,
                            skip_group_check=True,
                        )
                    # QK^T accumulated on top: S^T += K_j^T.T @ Q^T
                    kT_j = kT[:, c0 : c0 + 128]
                    for cs in range(0, span, 512):
                        ce = min(cs + 512, span)
                        nc.tensor.matmul(
                            st_ps[:, cs:ce],
                            _mm_dtype(kT_j, use_f32r),
                            _mm_dtype(qT[:, c0 + cs : c0 + ce], use_f32r),
                            start=False,
                            stop=(ce == span),
                            skip_group_check=True,
                        )

                    # P^T = exp(S^T): single ACT pass, PSUM -> SBUF
                    pT = p_pool.tile([128, span], F32, tag="pT")
                    nc.scalar.activation(
                        pT, st_ps, mybir.ActivationFunctionType.Exp
                    )

                    # causal mask on the diagonal block: zero where kj > qi
                    nc.gpsimd.affine_select(
                        out=pT[:, 0:128],
                        in_=pT[:, 0:128],
                        compare_op=mybir.AluOpType.is_ge,
                        fill=0.0,
                        base=0,
                        pattern=[[1, 128]],
                        channel_multiplier=-1,
                    )

                    # AV: out^T[:, c0:] += vhat_j.T @ P^T
                    for cs in range(0, span, 512):
                        ce = min(cs + 512, span)
                        nc.tensor.matmul(
                            outT_ps[:, c0 + cs : c0 + ce],
                            _mm_dtype(vhat[:, j, :], use_f32r),
                            _mm_dtype(pT[:, cs:ce], use_f32r),
                            start=(j == 0),
                            stop=(j == nt - 1),
                            skip_group_check=True,
                        )

                # ---- epilogue: transpose back, normalize, store -------
                outT_sb = out_pool.tile([d + 1, n], F32)
                nc.vector.tensor_copy(outT_sb, outT_ps)

                o2_ps = scratch_pool.tile([128, n], F32, tag="tscratch")
                for i in range(nt):
                    nc.tensor.transpose(
                        o2_ps[:, i * 128 : i * 128 + (d + 1)],
                        outT_sb[:, i * 128 : (i + 1) * 128],
                        ident[0 : d + 1, 0 : d + 1],
                    )
                o2_3d = o2_ps.rearrange("p (i c) -> p i c", c=128)

                recip = out_pool.tile([128, nt, 1], F32)
                nc.vector.reciprocal(recip, o2_3d[:, :, d : d + 1])

                out_sb = out_pool.tile([128, nt, d], F32)
                recip_b = bass.AP(
                    tensor=recip.tensor,
                    offset=recip.offset,
                    ap=[recip.ap[0], recip.ap[1], [0, d]],
                )
                nc.vector.tensor_mul(out_sb, o2_3d[:, :, 0:d], recip_b)

                nc.sync.dma_start(
                    out=out_dram[h].rearrange("(i p) d -> p i d", p=128),
                    in_=out_sb,
                )

    return nc


_BASS_CACHE = {}


def _get_bass(**kw):
    key = tuple(sorted(kw.items()))
    if key not in _BASS_CACHE:
        _BASS_CACHE[key] = build_attention_bass(**kw)
    return _BASS_CACHE[key]


def kernel(q, k, v, bias):
    """Full-input entry point: shard across 8 cores, run, gather."""
    from concourse.bass_utils import run_bass_kernel_spmd

    q = np.ascontiguousarray(np.asarray(q, dtype=np.float32)).reshape(B * H, N, D)
    k = np.ascontiguousarray(np.asarray(k, dtype=np.float32)).reshape(B * H, N, D)
    v = np.ascontiguousarray(np.asarray(v, dtype=np.float32)).reshape(B * H, N, D)
    bias = np.ascontiguousarray(np.asarray(bias, dtype=np.float32)).reshape(
        B * H, N, N
    )

    nc = _get_bass()
    hpc = HEADS_PER_CORE
    in_maps = [
        {
            "q": q[c * hpc : (c + 1) * hpc],
            "k": k[c * hpc : (c + 1) * hpc],
            "v": v[c * hpc : (c + 1) * hpc],
            "bias": bias[c * hpc : (c + 1) * hpc],
        }
        for c in range(N_CORES)
    ]
    res = run_bass_kernel_spmd(nc, in_maps, core_ids=list(range(N_CORES)))
    out = np.concatenate([r["out"] for r in res.results], axis=0)
    return out.reshape(B, H, N, D)


# revision 22
# speedup vs baseline: 1.3191x; 1.3191x over previous
"""Trainium2 Bass kernel for causal attention with bias.

Reference computation (per (b, h)):
    S = (Q @ K^T) * D**-0.5 + bias          # [N, N]
    S[j > i] = -FLT_MAX                     # causal mask
    P = softmax(S, axis=-1)
    out = P @ V                             # [N, D]

Shapes: B=4, H=16, N=1024, D=64, fp32. The 64 (b, h) pairs are sharded
8-per-core across 8 NeuronCores (head-parallel, no collectives).

Per-core kernel strategy (per head):
  - PE-transpose q, k into [D, N] layout (matmul against identity).
  - Loop over key blocks j (128 keys each). Exploit causality: only
    query blocks i >= j are computed.
  - The bias strip for (j, i>=j) is DMA'd in natural [qi, kj] layout and
    transposed by the PE into PSUM (is_transpose matmul, start=True).
    The QK^T matmul then ACCUMULATES on top (start=False), giving
    S^T = bias^T + K_j Q^T directly in PSUM, in transposed [kj, qi]
    layout.
  - Softmax skips the row-max subtraction (scores are O(10), fp32 exp
    has massive headroom), so exp(S^T) on the scalar engine is a single
    pass that also serves as the PSUM->SBUF move, yielding P^T in
    exactly the layout the AV matmul needs as its stationary/moving
    operand (contraction over keys).
  - The causal diagonal block is masked AFTER the exp by zeroing the
    strictly-lower triangle (kj > qi) with gpsimd affine_select.
  - V is augmented with a ones column; AV accumulates out^T[d, qi] and
    row 64 accumulates the softmax denominator for free.
  - PE transposes out^T back, DVE computes reciprocal of the
    denominators and scales, result DMA'd out.
"""

import numpy as np

import concourse.bass as bass
import concourse.mybir as mybir
import concourse.tile as tile
from concourse import bacc
from concourse.masks import make_identity

N_CORES = 8
B, H, N, D = 4, 16, 1024, 64
HEADS_PER_CORE = (B * H) // N_CORES  # 8
SCALE = float(D) ** -0.5

F32 = mybir.dt.float32
F32R = mybir.dt.float32r


def build_attention_bass(
    heads=HEADS_PER_CORE,
    n=N,
    d=D,
    use_f32r=True,
    qk_f32r=None,
    av_f32r=None,
    repeat=1,
):
    """Build the per-core Bass module. All cores run the same program
    (SPMD) on their own head slice."""
    nt = n // 128  # number of 128-row/col blocks
    assert n % 128 == 0 and d == 64
    # float32r matmuls run 4x faster than float32 (1 cycle/row for
    # moving dim >= 256); operands must be produced pre-rounded.
    if qk_f32r is None:
        qk_f32r = use_f32r
    if av_f32r is None:
        av_f32r = use_f32r
    QKDT = F32R if qk_f32r else F32
    MMDT = F32R if av_f32r else F32

    nc = bacc.Bacc("TRN2", target_bir_lowering=False)

    q_dram = nc.dram_tensor("q", [heads, n, d], F32, kind="ExternalInput")
    k_dram = nc.dram_tensor("k", [heads, n, d], F32, kind="ExternalInput")
    v_dram = nc.dram_tensor("v", [heads, n, d], F32, kind="ExternalInput")
    bias_dram = nc.dram_tensor("bias", [heads, n, n], F32, kind="ExternalInput")
    out_dram = nc.dram_tensor("out", [heads, n, d], F32, kind="ExternalOutput")

    with tile.TileContext(nc) as tc:
        with (
            tc.tile_pool(name="const", bufs=1) as const_pool,
            tc.tile_pool(name="qk", bufs=2) as qk_pool,
            tc.tile_pool(name="biasp", bufs=4) as bias_pool,
            tc.tile_pool(name="pt", bufs=2) as p_pool,
            tc.tile_pool(name="outp", bufs=2) as out_pool,
            tc.tile_pool(name="st", bufs=2, space="PSUM") as st_pool,
            tc.tile_pool(name="acc", bufs=1, space="PSUM") as acc_pool,
            tc.tile_pool(name="scratch", bufs=1, space="PSUM") as scratch_pool,
        ):
            ident = const_pool.tile([128, 128], F32)
            make_identity(nc, ident)

            import contextlib

            rep_ctx = (
                tc.For_i(0, repeat, 1) if repeat > 1 else contextlib.nullcontext()
            )
            with rep_ctx:
                body(
                    nc,
                    tc,
                    heads,
                    n,
                    d,
                    nt,
                    QKDT,
                    MMDT,
                    q_dram,
                    k_dram,
                    v_dram,
                    bias_dram,
                    out_dram,
                    ident,
                    qk_pool,
                    bias_pool,
                    p_pool,
                    out_pool,
                    st_pool,
                    acc_pool,
                    scratch_pool,
                )

    # bacc lowering: register alloc, nop fusion, and splitting sync waits
    # to satisfy the 1-wait-per-instruction TRN2 constraint.
    nc.compile()
    return nc


def body(
    nc,
    tc,
    heads,
    n,
    d,
    nt,
    QKDT,
    MMDT,
    q_dram,
    k_dram,
    v_dram,
    bias_dram,
    out_dram,
    ident,
    qk_pool,
    bias_pool,
    p_pool,
    out_pool,
    st_pool,
    acc_pool,
    scratch_pool,
):
    F32 = mybir.dt.float32
    F32R = mybir.dt.float32r
    if True:
        if True:
            for h in range(heads):
                # ---- load q, k, v (natural layout) --------------------
                q_nat = qk_pool.tile([128, nt, d], F32, tag="nat")
                nc.sync.dma_start(
                    out=q_nat, in_=q_dram[h].rearrange("(t p) d -> p t d", p=128)
                )
                k_nat = qk_pool.tile([128, nt, d], F32, tag="nat2")
                nc.sync.dma_start(
                    out=k_nat, in_=k_dram[h].rearrange("(t p) d -> p t d", p=128)
                )
                # v with an extra ones column (denominator trick)
                vhat = qk_pool.tile([128, nt, d + 1], MMDT)
                v_src = v_dram[h].rearrange("(t p) d -> p t d", p=128)
                if MMDT is F32R:
                    v_src = v_src.bitcast(F32R)
                nc.sync.dma_start(out=vhat[:, :, 0:d], in_=v_src)
                nc.gpsimd.memset(vhat[:, :, d : d + 1].bitcast(F32), 1.0)

                # ---- transpose q, k into [d, n] -----------------------
                qT_ps = scratch_pool.tile([d, n], F32, tag="tscratch")
                for t in range(nt):
                    nc.tensor.transpose(
                        qT_ps[:, t * 128 : (t + 1) * 128], q_nat[:, t, :], ident
                    )
                qT = qk_pool.tile([d, n], QKDT)
                # fold the attention scale into q while evacuating PSUM
                nc.vector.tensor_scalar_mul(qT, qT_ps, SCALE)

                kT_ps = scratch_pool.tile([d, n], F32, tag="tscratch")
                for t in range(nt):
                    nc.tensor.transpose(
                        kT_ps[:, t * 128 : (t + 1) * 128], k_nat[:, t, :], ident
                    )
                kT = qk_pool.tile([d, n], QKDT)
                nc.vector.tensor_copy(kT, kT_ps)

                # out^T accumulator [d+1, n]; row d is the denominator
                outT_ps = acc_pool.tile([d + 1, n], F32)

                # ---- main loop over key blocks j ----------------------
                for j in range(nt):
                    span = (nt - j) * 128  # query columns j*128 .. n
                    c0 = j * 128  # absolute first query column

                    # bias strip: natural [qi, i-block, kj] layout
                    bias_sb = bias_pool.tile([128, nt - j, 128], F32, tag="bias_sb")
                    nc.sync.dma_start(
                        out=bias_sb,
                        in_=bias_dram[h][c0:n, c0 : c0 + 128].rearrange(
                            "(i p) c -> p i c", p=128
                        ),
                    )

                    # S^T strip in PSUM: [kj, qi] for qi in [c0, n)
                    st_ps = st_pool.tile([128, span], F32, tag="st")

                    # QK^T first: S^T = K_j^T.T @ Q^T. Each 512-col chunk
                    # covers a full PSUM bank, so start=True gives the
                    # whole bank a uniform has_written state.
                    kT_j = kT[:, c0 : c0 + 128]
                    for cs in range(0, span, 512):
                        ce = min(cs + 512, span)
                        nc.tensor.matmul(
                            st_ps[:, cs:ce],
                            kT_j,
                            qT[:, c0 + cs : c0 + ce],
                            start=True,
                            stop=False,
                            skip_group_check=True,
                        )
                    # bias^T accumulated on top via PE transpose
                    for ii in range(nt - j):
                        nc.tensor.matmul(
                            st_ps[:, ii * 128 : (ii + 1) * 128],
                            bias_sb[:, ii, :],
                            ident,
                            is_transpose=True,
                            start=False,
                            stop=(ii == nt - j - 1),
                            skip_group_check=True,
                        )

                    # P^T = exp(S^T): single ACT pass, PSUM -> SBUF
                    pT = p_pool.tile([128, span], MMDT, tag="pT")
                    nc.scalar.activation(
                        pT, st_ps, mybir.ActivationFunctionType.Exp
                    )

                    # causal mask on the diagonal block: zero where kj > qi
                    nc.gpsimd.affine_select(
                        out=pT[:, 0:128],
                        in_=pT[:, 0:128],
                        compare_op=mybir.AluOpType.is_ge,
                        fill=0.0,
                        base=0,
                        pattern=[[1, 128]],
                        channel_multiplier=-1,
                    )

                    # AV: out^T[:, c0:] += vhat_j.T @ P^T
                    for cs in range(0, span, 512):
                        ce = min(cs + 512, span)
                        nc.tensor.matmul(
                            outT_ps[:, c0 + cs : c0 + ce],
                            vhat[:, j, :],
                            pT[:, cs:ce],
                            start=(j == 0),
                            stop=(j == nt - 1),
                            skip_group_check=True,
                        )

                # ---- epilogue: transpose back, normalize, store -------
                outT_sb = out_pool.tile([d + 1, n], F32)
                nc.vector.tensor_copy(outT_sb, outT_ps)

                o2_ps = scratch_pool.tile([128, n], F32, tag="tscratch")
                for i in range(nt):
                    nc.tensor.transpose(
                        o2_ps[:, i * 128 : i * 128 + (d + 1)],
                        outT_sb[:, i * 128 : (i + 1) * 128],
                        ident[0 : d + 1, 0 : d + 1],
                    )
                o2_3d = o2_ps.rearrange("p (i c) -> p i c", c=128)

                recip = out_pool.tile([128, nt, 1], F32)
                nc.vector.reciprocal(recip, o2_3d[:, :, d : d + 1])

                out_sb = out_pool.tile([128, nt, d], F32)
                recip_b = bass.AP(
                    tensor=recip.tensor,
                    offset=recip.offset,
                    ap=[recip.ap[0], recip.ap[1], [0, d]],
                )
                nc.vector.tensor_mul(out_sb, o2_3d[:, :, 0:d], recip_b)

                nc.sync.dma_start(
                    out=out_dram[h].rearrange("(i p) d -> p i d", p=128),
                    in_=out_sb,
                )


_BASS_CACHE = {}


def _get_bass(**kw):
    key = tuple(sorted(kw.items()))
    if key not in _BASS_CACHE:
        _BASS_CACHE[key] = build_attention_bass(**kw)
    return _BASS_CACHE[key]


def kernel(q, k, v, bias):
    """Full-input entry point: shard across 8 cores, run, gather."""
    from concourse.bass_utils import run_bass_kernel_spmd

    q = np.ascontiguousarray(np.asarray(q, dtype=np.float32)).reshape(B * H, N, D)
    k = np.ascontiguousarray(np.asarray(k, dtype=np.float32)).reshape(B * H, N, D)
    v = np.ascontiguousarray(np.asarray(v, dtype=np.float32)).reshape(B * H, N, D)
    bias = np.ascontiguousarray(np.asarray(bias, dtype=np.float32)).reshape(
        B * H, N, N
    )

    nc = _get_bass()
    hpc = HEADS_PER_CORE
    in_maps = [
        {
            "q": q[c * hpc : (c + 1) * hpc],
            "k": k[c * hpc : (c + 1) * hpc],
            "v": v[c * hpc : (c + 1) * hpc],
            "bias": bias[c * hpc : (c + 1) * hpc],
        }
        for c in range(N_CORES)
    ]
    res = run_bass_kernel_spmd(nc, in_maps, core_ids=list(range(N_CORES)))
    out = np.concatenate([r["out"] for r in res.results], axis=0)
    return out.reshape(B, H, N, D)
